# revision 5
# baseline (speedup 1.0000x reference)
"""Bass TRN2 kernel v2 for the boundary cosine-similarity context loss.

Per core (8 cores): batch b = k//2, row-half h = k%2; 190 produced rows in
19 blocks x 10 rows. Host pre-converts features to bf16.

Row mapping inside a block: row = 64*w + 5*s + r5 for wave w in {0,1}
(produced rows 5w..5w+4), slot s in 0..11 (shift index SHIFTS[s]), r5 in
0..4. Rows 60..63 and 124..127 are dead (never used by host).

Per block n (y0 = 2+10n, win0 = y0*W):
  g [C, 4616] bf16 <- x[:, win0 : win0+4616]  (direct HBM DMA)
  sq = g[:, :4608]^2 (ACT, prefetched one block early)
  norms: 12 eye12 matmuls -> n2 psum [12, 384]; Ln -> Exp(-.5) -> inv12;
         ACT copy n2 -> n2sb (for the square-trick correction)
  products (per wave, in0 = g[:, base:base+1920] broadcast):
    slots 2..11 in ONE DVE TensorTensor per wave: 4-dim AP over the
    [dy x dx] window grid (strides 384, 1), stride-0 broadcast in0;
    slots 0,1 via square trick: a = g + g_shift (Pool), t = a^2 (ACT);
    pd accumulates S - n2p - n2q = 2*dot via -1 h-matmuls (host folds
    lab *= 2, ww /= 4 for these slots).
  dots: per wave, one psum accumulation group of 64 rows (eye64 one-hot
        matmuls; psum output base must be 0/64).
  ip/iq: eq selection matmuls from inv12 -> psum.
  post (copies emitted next block, reduce two blocks later):
    pk=copy(pd); ipsb=copy(ipp); iqsb=copy(iqp) on ACT; Pool chain
    z = pk*ipsb*iqsb; v = z - lab; w1 = v*ww; w2 = w1*v; DVE
    acc[:, n] = reduce_sum(w2).
Host folds 1/cnt, 1/24, valid, 1/n_valid.
"""

import numpy as np
import ml_dtypes

import concourse.bass as bass
import concourse.mybir as mybir
from concourse.tile import TileContext
from concourse.vector_clock import ScopedClock
from concourse import bass_utils

BF16 = ml_dtypes.bfloat16
F32 = mybir.dt.float32
BF = mybir.dt.bfloat16

W = 384
C = 128
SHIFTS = [(0, 1), (0, 2),
          (1, -2), (1, -1), (1, 0), (1, 1), (1, 2),
          (2, -2), (2, -1), (2, 0), (2, 1), (2, 2)]
OFFS = [dy * W + dx for dy, dx in SHIFTS]
NSH = 12
RPB = 10
NROWS = 128            # padded rows: 64*w + 5*s + r5; dead 60..63/124..127
SQSLOTS = (0, 1)       # slots via the square trick (dy=0)
# merged DVE products, one op per (wave, dy): 5 dx-windows each, stride 1.
# dy=1: offs 382..386 -> slots 2..6; dy=2: offs 766..770 -> slots 7..11.
# Splitting by dy lets PE start the dy=1 dot rows before dy=2 products land.
DVE_GROUPS = [(2, 5, 382), (7, 5, 766)]
GLEN = 4616
ODLEN = 4612
NBLK = 19
NPIX = 194 * W
XPAD = 16


def _patch_tile_drain():
    if getattr(TileContext, "_drain_patched", False):
        return

    def _drain_and_barrier(self, tick_clock, wait_clock):
        drain_inst = self.nc.sync.drain()
        wait_clock.add_sem_waits(
            drain_inst.ins, ScopedClock({None: tick_clock.global_clock}))
        si = drain_inst.ins.sync_info
        if si is not None and si.on_wait and len(si.on_wait) > 1:
            waits = list(si.on_wait)
            drain_inst.ins.sync_info = mybir.SyncInfo(
                on_wait=[waits[-1]], on_update=list(si.on_update or []))
            for w in waits[:-1]:
                nop = self.nc.sync.nop(nofuse=True)
                nop.ins.sync_info = mybir.SyncInfo(on_wait=[w], on_update=[])
        self.nc.all_engine_barrier()
        popped = self.nc._tile_sem_poison_stack.pop()
        assert popped is self._sem_poison
        self.nc.clear_and_free_semaphores(list(self.sems.allocated().values()))
        self.nc.all_engine_barrier()

    TileContext._drain_and_barrier = _drain_and_barrier
    TileContext._drain_patched = True


_WSPLIT_N = [0]


def _split_multi_waits(nc, max_waits=1):
    """This container's walrus rejects instructions with more than one sync
    wait; hoist excess waits onto same-engine NOPs inserted just before."""
    for fn in nc.m.functions:
        for blk in fn.blocks:
            insts = blk.instructions
            out = []
            for inst in insts:
                si = inst.sync_info
                if si is not None and si.on_wait and len(si.on_wait) > max_waits:
                    waits = list(si.on_wait)
                    keep = waits[-max_waits:]
                    for w in waits[:-max_waits]:
                        _WSPLIT_N[0] += 1
                        nop = mybir.InstNoOp(
                            name=f"wsplit_{_WSPLIT_N[0]}", ins=[], outs=[])
                        nop.engine = inst.engine
                        nop.sync_info = mybir.SyncInfo(on_wait=[w],
                                                       on_update=[])
                        out.append(nop)
                    inst.sync_info = mybir.SyncInfo(
                        on_wait=keep, on_update=list(si.on_update or []))
                out.append(inst)
            blk.instructions = out


def _ap3(t, p_ap, off, d1, n1, d2, n2):
    return bass.AP(t.tensor, t.offset + off, [p_ap, [d1, n1], [d2, n2]])


def build_nc(nblk=NBLK, repeat=1):
    _patch_tile_drain()
    nc = bass.Bass()
    x = nc.dram_tensor("x", [C, NPIX + XPAD], BF, kind="ExternalInput")
    labw = nc.dram_tensor("labw", [nblk, NROWS, 2 * W], BF,
                          kind="ExternalInput")
    eye12 = nc.dram_tensor("eye12", [C, NSH, NSH], BF, kind="ExternalInput")
    eye64 = nc.dram_tensor("eye64", [C, 64, 64], BF, kind="ExternalInput")
    eq = nc.dram_tensor("eq", [NSH, 6, NROWS], BF, kind="ExternalInput")
    hwt = nc.dram_tensor("hwt", [NSH, 6, 64], BF, kind="ExternalInput")
    out = nc.dram_tensor("out", [NROWS, nblk], F32, kind="ExternalOutput")

    with TileContext(nc) as tc:
        with (tc.tile_pool(name="const", bufs=1) as cpool,
              tc.tile_pool(name="gbuf", bufs=2) as gpool,
              tc.tile_pool(name="sqp", bufs=2) as sqpool,
              tc.tile_pool(name="tp", bufs=2) as tpool,
              tc.tile_pool(name="apl", bufs=4) as apool,
              tc.tile_pool(name="inv", bufs=2) as ipool,
              tc.tile_pool(name="post", bufs=2) as postpool,
              tc.tile_pool(name="npsum", bufs=2, space="PSUM") as npsum,
              tc.tile_pool(name="dpsum", bufs=2, space="PSUM") as dpsum,
              tc.tile_pool(name="ppsum", bufs=2, space="PSUM") as ppsum,
              tc.tile_pool(name="qpsum", bufs=2, space="PSUM") as qpsum):

            eye12_sb = cpool.tile([C, NSH, NSH], BF)
            nc.sync.dma_start(eye12_sb[:], eye12[:])
            eye64_sb = cpool.tile([C, 64, 64], BF)
            nc.sync.dma_start(eye64_sb[:], eye64[:])
            eq_sb = cpool.tile([NSH, 6, NROWS], BF)
            nc.sync.dma_start(eq_sb[:], eq[:])
            hw_sb = cpool.tile([NSH, 6, 64], BF)
            nc.sync.dma_start(hw_sb[:], hwt[:])
            acc = cpool.tile([NROWS, nblk], F32)

            def load_g(n):
                win0 = (2 + RPB * n) * W
                g = gpool.tile([C, GLEN], BF, tag="g", name=f"g{n}")
                nc.sync.dma_start(g[:], x[:, win0:win0 + GLEN])
                return g, None

            def square(g, n):
                sq = sqpool.tile([C, 12 * W], BF, tag="sq", name=f"sq{n}")
                nc.scalar.square(sq[:], g[:, 0:12 * W])
                return sq

            g, godd = load_g(0)
            sq = square(g, 0)
            prev = None
            pool_post = None
            pending_reduce = None

            total = nblk * repeat
            for it in range(total):
                n = it % nblk
                first = it <= 1

                # ---- ACT copies for the previous block (unblock Pool) ----
                if prev is not None:
                    pool_post = _emit_copies(nc, postpool, *prev)
                    prev = None

                # ---- Pool adds + DVE grouped products ----
                tw = []
                for w in range(2):
                    base = w * 5 * W
                    t = tpool.tile([C, NSH, 5 * W], BF, tag="t",
                                   name=f"t{it}_{w}")
                    tw.append(t)
                    for s0, nw, foff in DVE_GROUPS:
                        p_g = g[:].ap[0]
                        in0b = bass.AP(g.tensor, g.offset + base,
                                       [p_g, [0, nw], [1, 5 * W]])
                        in1 = bass.AP(g.tensor, g.offset + base + foff,
                                      [p_g, [1, nw], [1, 5 * W]])
                        ot = bass.AP(t.tensor, t.offset + s0 * 5 * W,
                                     [t[:].ap[0], [5 * W, nw], [1, 5 * W]])
                        nc.vector.tensor_mul(ot, in0b, in1)
                    for s in SQSLOTS:
                        off = OFFS[s]
                        in1 = g[:, base + off:base + off + 5 * W]
                        a = apool.tile([C, 5 * W], BF, tag="a",
                                       name=f"a{it}_{w}_{s}")
                        nc.gpsimd.tensor_add(a[:],
                                             g[:, base:base + 5 * W], in1)
                        nc.scalar.square(t[:, s, :], a[:])

                # ---- norms ----
                n2 = npsum.tile([NSH, W], F32, tag="n2", name=f"n2_{it}")
                for j in range(12):
                    nc.tensor.matmul(n2[:], eye12_sb[:, j, :],
                                     sq[:, j * W:(j + 1) * W],
                                     start=(j == 0), stop=(j == 11))
                n2sb = ipool.tile([NSH, W + 4], BF, tag="n2sb",
                                  name=f"n2sb{it}")
                nc.scalar.copy(n2sb[:, 2:W + 2], n2[:])
                lnt = ipool.tile([NSH, W], F32, tag="lnt", name=f"lnt{it}")
                nc.scalar.activation(lnt[:], n2[:],
                                     mybir.ActivationFunctionType.Ln)
                inv12 = ipool.tile([NSH, W + 4], BF, tag="inv12",
                                   name=f"inv{it}")
                nc.scalar.activation(inv12[:, 2:W + 2], lnt[:],
                                     mybir.ActivationFunctionType.Exp,
                                     scale=-0.5)
                if first:
                    nc.vector.memset(inv12[:, 0:2], 0.0)
                    nc.vector.memset(inv12[:, W + 2:W + 4], 0.0)
                    nc.vector.memset(n2sb[:, 0:2], 0.0)
                    nc.vector.memset(n2sb[:, W + 2:W + 4], 0.0)

                # ---- dot matmuls ----
                pd = dpsum.tile([NROWS, W], F32, tag="pd", name=f"pd{it}")
                for w in range(2):
                    t = tw[w]
                    base_row = 64 * w
                    rows = list(range(10, 60)) + list(range(0, 10))
                    for i, row in enumerate(rows):
                        s, r5 = row // 5, row % 5
                        nc.tensor.matmul(
                            pd[base_row:base_row + 64, :],
                            eye64_sb[:, row, :],
                            t[:, s, r5 * W:(r5 + 1) * W],
                            start=(i == 0), stop=False)
                    for i, (hj, dxo) in enumerate([(0, 0), (1, 1), (2, 2)]):
                        nc.tensor.matmul(
                            pd[base_row:base_row + 64, :],
                            hw_sb[:, 3 * w + hj, :],
                            n2sb[:, 2 + dxo:2 + dxo + W],
                            start=False, stop=(i == 2))

                # ---- ip / iq selection matmuls ----
                ipp = ppsum.tile([NROWS, W], F32, tag="ipp", name=f"ipp{it}")
                nc.tensor.matmul(ipp[:], eq_sb[:, 0, :],
                                 inv12[:, 2:W + 2], start=True, stop=True)
                iqp = qpsum.tile([NROWS, W], F32, tag="iqp", name=f"iqp{it}")
                for di in range(5):
                    nc.tensor.matmul(iqp[:], eq_sb[:, 1 + di, :],
                                     inv12[:, di:di + W],
                                     start=(di == 0), stop=(di == 4))

                # ---- prefetch next block's g + sq ----
                if it + 1 < total:
                    g, godd = load_g((it + 1) % nblk)
                    sq = square(g, it + 1)

                # ---- deferred reduce (block it-2), then pool chain (it-1) ----
                if pending_reduce is not None:
                    w2p, np_ = pending_reduce
                    nc.vector.reduce_sum(acc[:, np_:np_ + 1], w2p[:],
                                         axis=mybir.AxisListType.X)
                    pending_reduce = None
                if pool_post is not None:
                    pending_reduce = _emit_pool_chain(nc, cpool, *pool_post)
                    pool_post = None
                lw = postpool.tile([NROWS, 2 * W], BF, tag="lw",
                                   name=f"lw{it}")
                nc.sync.dma_start(lw[:], labw[n])
                prev = (pd, ipp, iqp, lw, n)

            pool_post = _emit_copies(nc, postpool, *prev)
            if pending_reduce is not None:
                w2p, np_ = pending_reduce
                nc.vector.reduce_sum(acc[:, np_:np_ + 1], w2p[:],
                                     axis=mybir.AxisListType.X)
            w2p, np_ = _emit_pool_chain(nc, cpool, *pool_post)
            nc.vector.reduce_sum(acc[:, np_:np_ + 1], w2p[:],
                                 axis=mybir.AxisListType.X)
            nc.sync.dma_start(out[:], acc[:])
    _split_multi_waits(nc)
    return nc


def _emit_copies(nc, postpool, pd, ipp, iqp, lw, n):
    pk = postpool.tile([NROWS, W], BF, tag="pk", name=f"pk{n}")
    nc.scalar.copy(pk[:], pd[:])
    ipsb = postpool.tile([NROWS, W], BF, tag="ipsb", name=f"ipsb{n}")
    nc.scalar.copy(ipsb[:], ipp[:])
    iqsb = postpool.tile([NROWS, W], BF, tag="iqsb", name=f"iqsb{n}")
    nc.scalar.copy(iqsb[:], iqp[:])
    return (pk, ipsb, iqsb, lw, n)


def _emit_pool_chain(nc, cpool, pk, ipsb, iqsb, lw, n):
    z2 = cpool.tile([NROWS, W], BF, tag="z2", bufs=2, name=f"z2_{n}")
    nc.gpsimd.tensor_mul(z2[:], pk[:], ipsb[:])
    z = cpool.tile([NROWS, W], BF, tag="z", bufs=2, name=f"z{n}")
    nc.gpsimd.tensor_mul(z[:], z2[:], iqsb[:])
    v = cpool.tile([NROWS, W], BF, tag="v", bufs=2, name=f"v{n}")
    nc.gpsimd.tensor_sub(v[:], z[:], lw[:, 0:W])
    w1 = cpool.tile([NROWS, W], BF, tag="w1", bufs=2, name=f"w1_{n}")
    nc.gpsimd.tensor_mul(w1[:], v[:], lw[:, W:2 * W])
    w2 = cpool.tile([NROWS, W], BF, tag="w2", bufs=2, name=f"w2_{n}")
    nc.gpsimd.tensor_mul(w2[:], w1[:], v[:])
    return (w2, n)


def make_consts():
    eye12 = np.broadcast_to(np.eye(NSH, dtype=BF16), (C, NSH, NSH)).copy()
    eye64 = np.broadcast_to(np.eye(64, dtype=BF16), (C, 64, 64)).copy()
    # eq[d, j, row]: selection weights, row = 64w + 5s + r5, j = window row.
    eq = np.zeros((6, NSH, NROWS), BF16)
    for w in range(2):
        for s in range(NSH):
            dy, dx = SHIFTS[s]
            for r5 in range(5):
                row = 64 * w + 5 * s + r5
                j = 5 * w + r5
                eq[0, j, row] = 1                    # ip: inv at produced row
                eq[1 + (dx + 2), j + dy, row] = 1    # iq: inv at partner
    # hwm[w*3 + k, j, col]: -1 one-hots, group rows 64w..64w+63, col=5s+r5.
    hwm = np.zeros((6, NSH, 64), BF16)
    for w in range(2):
        for si, s in enumerate(SQSLOTS):
            dy, dx = SHIFTS[s]
            for r5 in range(5):
                col = 5 * s + r5
                j = 5 * w + r5
                hwm[3 * w + 0, j, col] = -1            # -n2 at produced pixel
                hwm[3 * w + 1 + si, j + dy, col] = -1  # -n2 at partner
    return eye12, eye64, np.ascontiguousarray(eq.transpose(1, 0, 2)), \
        np.ascontiguousarray(hwm.transpose(1, 0, 2))


def host_prep(er_input, seg_label, gt_boundary_seg, nblk=NBLK):
    B, _, H, Wd = er_input.shape
    f32 = np.float32
    gb = np.where(gt_boundary_seg == 255, 0, gt_boundary_seg)
    slc = np.where(seg_label == 255, 0, seg_label)
    gt_b1 = gb * slc[:, 1]
    boundary = gt_b1 > 0
    iy = np.arange(H)
    ix = np.arange(Wd)
    interior = (((iy >= 2) & (iy <= H - 3))[:, None]
                & ((ix >= 2) & (ix <= Wd - 3))[None, :])
    sel = boundary & interior
    cnt = sel.sum(axis=(1, 2)).astype(f32)
    valid = boundary.sum(axis=(1, 2)) >= 1
    n_valid = valid.astype(f32).sum()

    seg_f = seg_label.astype(f32)
    lab_stack = np.empty((NSH, B, H, Wd), f32)
    w_stack = np.empty((NSH, B, H, Wd), f32)
    sel_f = sel.astype(f32)
    for m, (dy, dx) in enumerate(SHIFTS):
        rolled = np.roll(seg_f, (-dy, -dx), axis=(2, 3))
        lab_stack[m] = (seg_f * rolled).sum(axis=1)
        sh = np.zeros_like(sel_f)
        ys0, ys1 = max(0, -dy), min(H, H - dy)
        xs0, xs1 = max(0, -dx), min(Wd, Wd - dx)
        sh[:, ys0:ys1, xs0:xs1] = sel_f[:, ys0 + dy:ys1 + dy,
                                        xs0 + dx:xs1 + dx]
        w_stack[m] = sel_f + sh
    # square-trick slots: kernel computes 2*dot -> z = 2*cos
    for s in SQSLOTS:
        lab_stack[s] *= 2.0
        w_stack[s] *= 0.25

    eye12, eye64, eqm, hwm = make_consts()
    per_core = []
    for k in range(8):
        b, h = k // 2, k % 2
        r0 = 0 if h == 0 else 190
        xs = np.zeros((C, NPIX + XPAD), BF16)
        xs[:, :NPIX] = er_input[b, :, r0:r0 + 194, :].reshape(C, -1)
        # lwc[n, 64w+5s+r5] <- (lab, ww)[s] at global row r0+2+10n+5w+r5
        gr = (r0 + 2 + RPB * np.arange(nblk)[:, None, None]
              + 5 * np.arange(2)[None, :, None]
              + np.arange(5)[None, None, :])          # [nblk, 2, 5]
        lab_g = lab_stack[:, b][:, gr]                # [NSH, nblk, 2, 5, W]
        ww_g = w_stack[:, b][:, gr]
        lwc = np.zeros((nblk, NROWS, 2 * Wd), BF16)
        rowidx = (64 * np.arange(2)[:, None, None]
                  + 5 * np.arange(NSH)[None, :, None]
                  + np.arange(5)[None, None, :]).reshape(-1)  # [2*12*5]
        lab_p = lab_g.transpose(1, 2, 0, 3, 4).reshape(nblk, 120, Wd)
        ww_p = ww_g.transpose(1, 2, 0, 3, 4).reshape(nblk, 120, Wd)
        lwc[:, rowidx, 0:Wd] = lab_p
        lwc[:, rowidx, Wd:2 * Wd] = ww_p
        per_core.append({"x": xs, "labw": lwc, "eye12": eye12,
                         "eye64": eye64, "eq": eqm, "hwt": hwm})
    return per_core, dict(cnt=cnt, valid=valid, n_valid=n_valid)


REAL_ROWS = np.array([r for r in range(NROWS) if (r % 64) < 60])


def finish(core_sums, meta):
    f32 = np.float32
    cnt, valid, n_valid = meta["cnt"], meta["valid"], meta["n_valid"]
    total = f32(0.0)
    for b in range(4):
        sb = f32(core_sums[2 * b] + core_sums[2 * b + 1])
        loss_b = sb / max(cnt[b], f32(1.0)) / f32(24.0)
        if valid[b]:
            total = total + loss_b
    total = total / max(n_valid, f32(1.0))
    if np.isnan(total):
        total = f32(0.0)
    return np.float32(total)


_NC_CACHE = {}


def kernel(er_input, seg_label, gt_boundary_seg):
    er_input = np.asarray(er_input)
    seg_label = np.asarray(seg_label)
    gt_boundary_seg = np.asarray(gt_boundary_seg)
    per_core, meta = host_prep(er_input, seg_label, gt_boundary_seg)
    if "nc" not in _NC_CACHE:
        _NC_CACHE["nc"] = build_nc()
    nc = _NC_CACHE["nc"]
    res = bass_utils.run_bass_kernel_spmd(nc, per_core,
                                          core_ids=list(range(8)))
    sums = [r["out"][REAL_ROWS].astype(np.float64).sum()
            for r in res.results]
    return finish(sums, meta)
